# revision 1
# baseline (speedup 1.0000x reference)
"""LoRA row-parallel linear on 8 TRN2 NeuronCores.

Problem: y = x @ W^T + delta, where per-token LoRA delta[t] = B[s] @ (A[s] @ x[t]),
s = token_to_slot[t] (8 adapters, rank 16, scaling baked into B).

Strategy: token data-parallel across the 8 cores (T=8192 -> 1024 tokens/core).
No collectives needed; each core computes its token block fully, in transposed
output space (y^T, un-transposed on the host):
  u^T   = A_all @ x_shard^T          (128 x T_SH; A_all = all 8 adapters stacked)
  uM^T  = u^T * mask^T               (one-hot select of each token's adapter)
  y^T   = W @ x^T + B_all^T @ uM^T   (PSUM accumulation: 32 k-tiles of W + 1 of B)
All matmuls run as float32r (TF32-like, FP22) at full PE rate with fp32
accumulate (measured ~227 ns per 128x128x512 matmul, the intrinsic pacing;
f32r matmuls are self-loading, so stationary-operand choice is neutral).

Schedule (per core): the first output-column block (ob0) runs its 32-k-tile
d-loop FIRST, so the PE has work while the 16 MB x^T shard streams in; the
u-pass (which needs the whole shard) runs after it, and ob0's LoRA delta is
applied as a separate accumulation + DVE add. Remaining obs fuse the delta as
a 33rd accumulation step.

Host prep: transposes x/W/A to put the contraction dim on partitions, builds
the one-hot mask from token_to_slot. Device does all the FLOPs.
"""

import numpy as np
import ml_dtypes

from concourse import bacc, tile, mybir
from concourse.bass_utils import run_bass_kernel_spmd
import concourse.bass_utils as _bu

# Disable S3 artifact upload in the trace path (no credentials in this container).
_bu.upload_artifacts = lambda tmpdir: "local://" + tmpdir

N_CORES = 8
T = 8192
D_IN = 4096
D_OUT = 4096
L = 8          # max adapters
R = 16         # max rank
LR = L * R     # 128 = stacked adapter dim
T_SH = T // N_CORES          # 1024 tokens per core
KT = D_IN // 128             # 32 contraction tiles
OB = D_OUT // 512            # 8 output-column superblocks
NO = 4                       # 128-wide output blocks per superblock
NT = T_SH // 512             # 2 token blocks (moving dim)

F32 = mybir.dt.float32
F32R = mybir.dt.float32r

_CACHED_NC = None


def _build():
    nc = bacc.Bacc("TRN2", target_bir_lowering=False, debug=False)

    xT_d = nc.dram_tensor("xT", [D_IN, T_SH], F32, kind="ExternalInput")
    wT_d = nc.dram_tensor("wT", [D_IN, D_OUT], F32, kind="ExternalInput")
    aT_d = nc.dram_tensor("aT", [D_IN, LR], F32, kind="ExternalInput")
    bC_d = nc.dram_tensor("bC", [LR, D_OUT], F32, kind="ExternalInput")
    mT_d = nc.dram_tensor("maskT", [LR, T_SH], mybir.dt.bfloat16, kind="ExternalInput")
    yT_d = nc.dram_tensor("yT", [D_OUT, T_SH], F32, kind="ExternalOutput")

    with tile.TileContext(nc) as tc:
        with (
            tc.tile_pool(name="resident", bufs=1) as rpool,
            tc.tile_pool(name="wstream", bufs=9) as wpool,
            tc.tile_pool(name="yout", bufs=3) as ypool,
            tc.tile_pool(name="psum", bufs=8, space="PSUM") as psum,
        ):
            # --- resident loads; xts interleaved with ob0's w tiles so the
            # --- ob0 d-loop can start as soon as the first k-tile lands.
            xts = []
            wts0 = []
            ats = []
            for d in range(KT):
                xt = rpool.tile([128, T_SH], F32R, tag=f"xt{d}")
                nc.sync.dma_start(xt[:], xT_d[d * 128:(d + 1) * 128, :].bitcast(F32R))
                xts.append(xt)
                wt = wpool.tile([128, 512], F32R, tag="wt", name=f"wt0_{d}")
                nc.sync.dma_start(wt[:], wT_d[d * 128:(d + 1) * 128, 0:512].bitcast(F32R))
                wts0.append(wt)
                at = rpool.tile([128, LR], F32R, tag=f"at{d}", name=f"at{d}")
                nc.sync.dma_start(at[:], aT_d[d * 128:(d + 1) * 128, :].bitcast(F32R))
                ats.append(at)
            bc = rpool.tile([LR, D_OUT], F32R, tag="bc")
            nc.sync.dma_start(bc[:], bC_d[:].bitcast(F32R))
            mask = rpool.tile([LR, T_SH], mybir.dt.bfloat16, tag="mask")
            nc.sync.dma_start(mask[:], mT_d[:])
            uTms = [rpool.tile([LR, 512], F32R, tag=f"uTm{ub}", name=f"uTm{ub}")
                    for ub in range(NT)]

            # --- phase 1: ob0 d-loop (base matmul only, no delta) --------------
            # psum tile (o, t) = y^T[o-block of 128, t-block of 512]
            pys0 = [[psum.tile([128, 512], F32, tag="acc", name=f"py0_{o}_{t}")
                     for t in range(NT)] for o in range(NO)]
            yo0s = {}
            for d in range(KT):
                for o in range(NO):
                    lw = wts0[d][:, o * 128:(o + 1) * 128]
                    for t in range(NT):
                        nc.tensor.matmul(
                            pys0[o][t][:], lw, xts[d][:, t * 512:(t + 1) * 512],
                            start=(d == 0), stop=(d == KT - 1), skip_group_check=True,
                        )
                        if d == KT - 1:
                            yo0 = rpool.tile([128, 512], F32, tag=f"yo0_{o}_{t}",
                                             name=f"yo0_{o}_{t}")
                            nc.vector.tensor_copy(yo0[:], pys0[o][t][:])
                            yo0s[o, t] = yo0

            # --- phase 2: u-pass (needs all xts, which have landed by now) -----
            for ub in range(NT):
                pu = psum.tile([128, 512], F32, tag="acc", name=f"pu{ub}")
                sl = slice(ub * 512, (ub + 1) * 512)
                for d in range(KT):
                    nc.tensor.matmul(
                        pu[:], ats[d][:], xts[d][:, sl],
                        start=(d == 0), stop=(d == KT - 1), skip_group_check=True,
                    )
                nc.vector.tensor_mul(uTms[ub][:], pu[:], mask[:, sl])

            # --- phase 3: ob0 delta + writeback --------------------------------
            for o in range(NO):
                for t in range(NT):
                    pd = psum.tile([128, 512], F32, tag="acc", name=f"pd{o}_{t}")
                    nc.tensor.matmul(
                        pd[:], bc[:, o * 128:(o + 1) * 128], uTms[t][:],
                        start=True, stop=True, skip_group_check=True,
                    )
                    yo = ypool.tile([128, 512], F32, tag="yo", name=f"yod{o}_{t}")
                    nc.vector.tensor_add(yo[:], yo0s[o, t][:], pd[:])
                    nc.sync.dma_start(
                        yT_d[o * 128:(o + 1) * 128, t * 512:(t + 1) * 512], yo[:])

            # --- phase 4: ob1..7 with fused delta ------------------------------
            for ob in range(1, OB):
                pys = [[psum.tile([128, 512], F32, tag="acc", name=f"py{ob}_{o}_{t}")
                        for t in range(NT)] for o in range(NO)]
                for d in range(KT):
                    wt = wpool.tile([128, 512], F32R, tag="wt", name=f"wt{ob}_{d}")
                    nc.sync.dma_start(
                        wt[:],
                        wT_d[d * 128:(d + 1) * 128,
                             ob * 512:(ob + 1) * 512].bitcast(F32R))
                    for o in range(NO):
                        lw = wt[:, o * 128:(o + 1) * 128]
                        og = ob * 512 + o * 128
                        for t in range(NT):
                            nc.tensor.matmul(
                                pys[o][t][:], lw, xts[d][:, t * 512:(t + 1) * 512],
                                start=(d == 0), stop=False, skip_group_check=True,
                            )
                            if d == KT - 1:
                                nc.tensor.matmul(
                                    pys[o][t][:], bc[:, og:og + 128], uTms[t][:],
                                    start=False, stop=True, skip_group_check=True,
                                )
                                yo = ypool.tile([128, 512], F32, tag="yo",
                                                name=f"yo{ob}_{o}_{t}")
                                nc.vector.tensor_copy(yo[:], pys[o][t][:])
                                nc.sync.dma_start(
                                    yT_d[og:og + 128, t * 512:(t + 1) * 512], yo[:])

    nc.compile()
    return nc


def _get_nc():
    global _CACHED_NC
    if _CACHED_NC is None:
        _CACHED_NC = _build()
    return _CACHED_NC


def _prep_in_maps(x, weight, lora_A, lora_B, token_to_slot):
    x = np.asarray(x, dtype=np.float32)
    weight = np.asarray(weight, dtype=np.float32)
    lora_A = np.asarray(lora_A, dtype=np.float32)
    lora_B = np.asarray(lora_B, dtype=np.float32)
    slots = np.asarray(token_to_slot)

    wT = np.ascontiguousarray(weight.T)                                    # [D_IN, D_OUT]
    aT = np.ascontiguousarray(lora_A.transpose(2, 0, 1).reshape(D_IN, LR))  # [D_IN, L*R]
    bC = np.ascontiguousarray(lora_B.transpose(0, 2, 1).reshape(LR, D_OUT)) # [L*R, D_OUT]

    # One-hot mask over stacked adapter rows; out-of-range slots -> all-zero.
    # bf16 is exact for 0/1 and halves the SBUF footprint.
    maskT = np.zeros((LR, T), dtype=np.float32)
    for l in range(L):
        maskT[l * R:(l + 1) * R, :] = (slots == l).astype(np.float32)[None, :]

    in_maps = []
    for c in range(N_CORES):
        tsl = slice(c * T_SH, (c + 1) * T_SH)
        in_maps.append({
            "xT": np.ascontiguousarray(x[tsl, :].T),
            "wT": wT,
            "aT": aT,
            "bC": bC,
            "maskT": np.ascontiguousarray(maskT[:, tsl]).astype(ml_dtypes.bfloat16),
        })
    return in_maps


def _run(inputs, trace=False, trace_cores=None):
    nc = _get_nc()
    in_maps = _prep_in_maps(**inputs)
    res = run_bass_kernel_spmd(
        nc, in_maps, core_ids=list(range(N_CORES)),
        trace=trace, trace_cores=trace_cores,
    )
    y = np.concatenate([res.results[c]["yT"].T for c in range(N_CORES)], axis=0)
    y = np.ascontiguousarray(y)
    return y, res


def _validate(inputs, y):
    """Cheap host-side sanity check: project y onto a random vector and compare
    with the host-computed projection. Catches the (rare, transient) device
    corruption observed on this setup; costs <1 s on host BLAS."""
    x = np.asarray(inputs["x"], dtype=np.float32)
    weight = np.asarray(inputs["weight"], dtype=np.float32)
    lora_A = np.asarray(inputs["lora_A"], dtype=np.float32)
    lora_B = np.asarray(inputs["lora_B"], dtype=np.float32)
    slots = np.asarray(inputs["token_to_slot"])

    rng = np.random.default_rng(12345)
    r = rng.standard_normal(D_OUT).astype(np.float64)

    base = x.astype(np.float64) @ (weight.astype(np.float64).T @ r)      # [T]
    aT = lora_A.transpose(2, 0, 1).reshape(D_IN, LR)                      # [D_IN, LR]
    bC = lora_B.transpose(0, 2, 1).reshape(LR, D_OUT)                     # [LR, D_OUT]
    u = (x @ aT).astype(np.float64)                                       # [T, LR]
    m = np.zeros((T, LR))
    for l in range(L):
        m[:, l * R:(l + 1) * R] = (slots == l).astype(np.float64)[:, None]
    exp = base + (u * m) @ (bC.astype(np.float64) @ r)                    # [T]
    got = y.astype(np.float64) @ r
    scale = np.abs(exp).max()
    rel = np.abs(got - exp).max() / scale
    return rel < 3e-3


def kernel(x, weight, lora_A, lora_B, token_to_slot):
    inputs = dict(x=x, weight=weight, lora_A=lora_A, lora_B=lora_B,
                  token_to_slot=token_to_slot)
    y = None
    for _attempt in range(3):
        y, _ = _run(inputs)
        if _validate(inputs, y):
            break
    return y



# revision 2
# speedup vs baseline: 1.0811x; 1.0811x over previous
"""LoRA row-parallel linear on 8 TRN2 NeuronCores.

Problem: y = x @ W^T + delta, where per-token LoRA delta[t] = B[s] @ (A[s] @ x[t]),
s = token_to_slot[t] (8 adapters, rank 16, scaling baked into B).

Strategy: token data-parallel across the 8 cores (T=8192 -> 1024 tokens/core).
No collectives needed; each core computes its token block fully, in transposed
output space (y^T, un-transposed on the host):
  u^T   = A_all @ x_shard^T          (128 x T_SH; A_all = all 8 adapters stacked)
  uM^T  = u^T * mask^T               (one-hot select of each token's adapter)
  y^T   = W @ x^T + B_all^T @ uM^T   (PSUM accumulation: 32 k-tiles of W + 1 of B)

All matmul operands are fp16 (host-converted): same PE streaming rate as
fp32r (1 moving column/cycle) but half the DMA bytes and FWL-eligible weight
loads (fp32 weights disable fast-weight-load, exposing ~11ns/MM of LDWEIGHTS).
Quantization error budget: fp16 has 10 mantissa bits; measured baseline error
1.5e-4 (fp32r); predicted ~4e-4 here vs the 2e-2 gate.

Schedule (per core): the first output-column block (ob0) runs its 32-k-tile
d-loop FIRST, so the PE has work while the 8 MB x^T shard streams in; the
u-pass (which needs the whole shard) runs after it, and ob0's LoRA delta is
applied as a separate accumulation + DVE add. Remaining obs fuse the delta as
a 33rd accumulation step.

Host prep: transposes x/W/A to put the contraction dim on partitions, tiles W
so each [128x512] block is contiguous in DRAM (single-descriptor DMA), builds
the one-hot mask from token_to_slot. Device does all the FLOPs; y returns as
fp16 y^T tiles, host reassembles + upcasts.
"""

import numpy as np

from concourse import bacc, tile, mybir
from concourse.bass_utils import run_bass_kernel_spmd
import concourse.bass_utils as _bu

# Disable S3 artifact upload in the trace path (no credentials in this container).
_bu.upload_artifacts = lambda tmpdir: "local://" + tmpdir

N_CORES = 8
T = 8192
D_IN = 4096
D_OUT = 4096
L = 8          # max adapters
R = 16         # max rank
LR = L * R     # 128 = stacked adapter dim
T_SH = T // N_CORES          # 1024 tokens per core
KT = D_IN // 128             # 32 contraction tiles
OB = D_OUT // 512            # 8 output-column superblocks
NO = 4                       # 128-wide output blocks per superblock
NT = T_SH // 512             # 2 token blocks (moving dim)

F32 = mybir.dt.float32
F16 = mybir.dt.float16

_CACHED_NC = None


def _build():
    nc = bacc.Bacc("TRN2", target_bir_lowering=False, debug=False)

    xT_d = nc.dram_tensor("xT", [D_IN, T_SH], F16, kind="ExternalInput")
    # w tiled [OB*KT*128, 512]: row block (ob*KT+d) is the [128,512] tile,
    # contiguous 128 KB in DRAM.
    wt_d = nc.dram_tensor("wTt", [OB * KT * 128, 512], F16, kind="ExternalInput")
    aT_d = nc.dram_tensor("aT", [D_IN, LR], F16, kind="ExternalInput")
    bC_d = nc.dram_tensor("bC", [LR, D_OUT], F16, kind="ExternalInput")
    mT_d = nc.dram_tensor("maskT", [LR, T_SH], F16, kind="ExternalInput")
    # y^T tiled [(o_block*NT + t)*128, 512] fp16, host reassembles.
    yt_d = nc.dram_tensor("yTt", [D_OUT * NT * 128 // 128, 512], F16,
                          kind="ExternalOutput")

    def wslice(ob, d):
        r = (ob * KT + d) * 128
        return wt_d[r:r + 128, :]

    def yslice(o128, t):
        r = (o128 * NT + t) * 128
        return yt_d[r:r + 128, :]

    with tile.TileContext(nc) as tc:
        with (
            tc.tile_pool(name="resident", bufs=1) as rpool,
            tc.tile_pool(name="wstream", bufs=9) as wpool,
            tc.tile_pool(name="yout", bufs=4) as ypool,
            tc.tile_pool(name="psum", bufs=8, space="PSUM") as psum,
        ):
            # --- resident loads; xts interleaved with ob0's w tiles so the
            # --- ob0 d-loop can start as soon as the first k-tile lands.
            xts = []
            wts0 = []
            ats = []
            for d in range(KT):
                xt = rpool.tile([128, T_SH], F16, tag=f"xt{d}")
                nc.sync.dma_start(xt[:], xT_d[d * 128:(d + 1) * 128, :])
                xts.append(xt)
                wt = wpool.tile([128, 512], F16, tag="wt", name=f"wt0_{d}")
                nc.sync.dma_start(wt[:], wslice(0, d))
                wts0.append(wt)
                at = rpool.tile([128, LR], F16, tag=f"at{d}", name=f"at{d}")
                nc.sync.dma_start(at[:], aT_d[d * 128:(d + 1) * 128, :])
                ats.append(at)
            bc = rpool.tile([LR, D_OUT], F16, tag="bc")
            nc.sync.dma_start(bc[:], bC_d[:])
            mask = rpool.tile([LR, T_SH], F16, tag="mask")
            nc.sync.dma_start(mask[:], mT_d[:])
            uTms = [rpool.tile([LR, 512], F16, tag=f"uTm{ub}", name=f"uTm{ub}")
                    for ub in range(NT)]

            # --- phase 1: ob0 d-loop (base matmul only, no delta) --------------
            # psum tile (o, t) = y^T[o-block of 128, t-block of 512]
            pys0 = [[psum.tile([128, 512], F32, tag="acc", name=f"py0_{o}_{t}")
                     for t in range(NT)] for o in range(NO)]
            yo0s = {}
            for d in range(KT):
                for o in range(NO):
                    lw = wts0[d][:, o * 128:(o + 1) * 128]
                    for t in range(NT):
                        nc.tensor.matmul(
                            pys0[o][t][:], lw, xts[d][:, t * 512:(t + 1) * 512],
                            start=(d == 0), stop=(d == KT - 1), skip_group_check=True,
                        )
                        if d == KT - 1:
                            yo0 = rpool.tile([128, 512], F32, tag=f"yo0_{o}_{t}",
                                             name=f"yo0_{o}_{t}")
                            nc.vector.tensor_copy(yo0[:], pys0[o][t][:])
                            yo0s[o, t] = yo0

            # --- phase 2: u-pass (needs all xts, which have landed by now) -----
            for ub in range(NT):
                pu = psum.tile([128, 512], F32, tag="acc", name=f"pu{ub}")
                sl = slice(ub * 512, (ub + 1) * 512)
                for d in range(KT):
                    nc.tensor.matmul(
                        pu[:], ats[d][:], xts[d][:, sl],
                        start=(d == 0), stop=(d == KT - 1), skip_group_check=True,
                    )
                nc.vector.tensor_mul(uTms[ub][:], pu[:], mask[:, sl])

            # --- phase 3: ob0 delta + writeback --------------------------------
            for o in range(NO):
                for t in range(NT):
                    pd = psum.tile([128, 512], F32, tag="acc", name=f"pd{o}_{t}")
                    nc.tensor.matmul(
                        pd[:], bc[:, o * 128:(o + 1) * 128], uTms[t][:],
                        start=True, stop=True, skip_group_check=True,
                    )
                    yo = ypool.tile([128, 512], F16, tag="yo", name=f"yod{o}_{t}")
                    nc.vector.tensor_add(yo[:], yo0s[o, t][:], pd[:])
                    nc.sync.dma_start(yslice(o, t), yo[:])

            # --- phase 4: ob1..7 with fused delta ------------------------------
            for ob in range(1, OB):
                pys = [[psum.tile([128, 512], F32, tag="acc", name=f"py{ob}_{o}_{t}")
                        for t in range(NT)] for o in range(NO)]
                for d in range(KT):
                    wt = wpool.tile([128, 512], F16, tag="wt", name=f"wt{ob}_{d}")
                    nc.sync.dma_start(wt[:], wslice(ob, d))
                    for o in range(NO):
                        lw = wt[:, o * 128:(o + 1) * 128]
                        og = ob * 512 + o * 128
                        for t in range(NT):
                            nc.tensor.matmul(
                                pys[o][t][:], lw, xts[d][:, t * 512:(t + 1) * 512],
                                start=(d == 0), stop=False, skip_group_check=True,
                            )
                            if d == KT - 1:
                                nc.tensor.matmul(
                                    pys[o][t][:], bc[:, og:og + 128], uTms[t][:],
                                    start=False, stop=True, skip_group_check=True,
                                )
                                yo = ypool.tile([128, 512], F16, tag="yo",
                                                name=f"yo{ob}_{o}_{t}")
                                nc.vector.tensor_copy(yo[:], pys[o][t][:])
                                nc.sync.dma_start(yslice(ob * 4 + o, t), yo[:])

    nc.compile()
    return nc


def _get_nc():
    global _CACHED_NC
    if _CACHED_NC is None:
        _CACHED_NC = _build()
    return _CACHED_NC


def _prep_in_maps(x, weight, lora_A, lora_B, token_to_slot):
    x = np.asarray(x, dtype=np.float32)
    weight = np.asarray(weight, dtype=np.float32)
    lora_A = np.asarray(lora_A, dtype=np.float32)
    lora_B = np.asarray(lora_B, dtype=np.float32)
    slots = np.asarray(token_to_slot)

    # W tiled: wTt[(ob*KT+d)*128 + i, j] = W^T[d*128+i, ob*512+j]
    # = weight[ob*512+j, d*128+i].  Build via reshape/transpose:
    # weight [D_OUT, D_IN] -> [OB, 512, KT, 128] -> [OB, KT, 128, 512]
    wTt = np.ascontiguousarray(
        weight.reshape(OB, 512, KT, 128).transpose(0, 2, 3, 1)
    ).reshape(OB * KT * 128, 512).astype(np.float16)
    aT = np.ascontiguousarray(
        lora_A.transpose(2, 0, 1).reshape(D_IN, LR)).astype(np.float16)
    bC = np.ascontiguousarray(
        lora_B.transpose(0, 2, 1).reshape(LR, D_OUT)).astype(np.float16)

    # One-hot mask over stacked adapter rows; out-of-range slots -> all-zero.
    maskT = np.zeros((LR, T), dtype=np.float16)
    for l in range(L):
        maskT[l * R:(l + 1) * R, :] = (slots == l).astype(np.float16)[None, :]

    in_maps = []
    for c in range(N_CORES):
        tsl = slice(c * T_SH, (c + 1) * T_SH)
        in_maps.append({
            "xT": np.ascontiguousarray(x[tsl, :].T.astype(np.float16)),
            "wTt": wTt,
            "aT": aT,
            "bC": bC,
            "maskT": np.ascontiguousarray(maskT[:, tsl]),
        })
    return in_maps


def _run(inputs, trace=False, trace_cores=None):
    nc = _get_nc()
    in_maps = _prep_in_maps(**inputs)
    res = run_bass_kernel_spmd(
        nc, in_maps, core_ids=list(range(N_CORES)),
        trace=trace, trace_cores=trace_cores,
    )
    # yTt rows: [(o128*NT + t)*128 + i, j] = y^T[o128*128+i, t*512+j]
    parts = []
    for c in range(N_CORES):
        yt = res.results[c]["yTt"].reshape(D_OUT // 128, NT, 128, 512)
        ycT = yt.transpose(0, 2, 1, 3).reshape(D_OUT, T_SH)   # y^T [D_OUT, T_SH]
        parts.append(ycT.T)
    y = np.concatenate(parts, axis=0).astype(np.float32)
    y = np.ascontiguousarray(y)
    return y, res


def _validate(inputs, y):
    """Cheap host-side sanity check: project y onto a random vector and compare
    with the host-computed projection. Catches the (rare, transient) device
    corruption observed on this setup; costs <1 s on host BLAS."""
    x = np.asarray(inputs["x"], dtype=np.float32)
    weight = np.asarray(inputs["weight"], dtype=np.float32)
    lora_A = np.asarray(inputs["lora_A"], dtype=np.float32)
    lora_B = np.asarray(inputs["lora_B"], dtype=np.float32)
    slots = np.asarray(inputs["token_to_slot"])

    rng = np.random.default_rng(12345)
    r = rng.standard_normal(D_OUT).astype(np.float64)

    base = x.astype(np.float64) @ (weight.astype(np.float64).T @ r)      # [T]
    aT = lora_A.transpose(2, 0, 1).reshape(D_IN, LR)                      # [D_IN, LR]
    bC = lora_B.transpose(0, 2, 1).reshape(LR, D_OUT)                     # [LR, D_OUT]
    u = (x @ aT).astype(np.float64)                                       # [T, LR]
    m = np.zeros((T, LR))
    for l in range(L):
        m[:, l * R:(l + 1) * R] = (slots == l).astype(np.float64)[:, None]
    exp = base + (u * m) @ (bC.astype(np.float64) @ r)                    # [T]
    got = y.astype(np.float64) @ r
    scale = np.abs(exp).max()
    rel = np.abs(got - exp).max() / scale
    return rel < 3e-3


def kernel(x, weight, lora_A, lora_B, token_to_slot):
    inputs = dict(x=x, weight=weight, lora_A=lora_A, lora_B=lora_B,
                  token_to_slot=token_to_slot)
    y = None
    for _attempt in range(3):
        y, _ = _run(inputs)
        if _validate(inputs, y):
            break
    return y


# revision 5
# speedup vs baseline: 1.1100x; 1.0268x over previous
"""LoRA row-parallel linear on 8 TRN2 NeuronCores.

Problem: y = x @ W^T + delta, where per-token LoRA delta[t] = B[s] @ (A[s] @ x[t]),
s = token_to_slot[t] (8 adapters, rank 16, scaling baked into B).

Strategy: token data-parallel across the 8 cores (T=8192 -> 1024 tokens/core).
No collectives needed; each core computes its token block fully, in transposed
output space (y^T, un-transposed on the host):
  u^T   = A_all @ x_shard^T          (128 x T_SH; A_all = all 8 adapters stacked)
  uM^T  = u^T * mask^T               (one-hot select of each token's adapter)
  y^T   = W @ x^T + B_all^T @ uM^T   (PSUM accumulation: 32 k-tiles of W + 1 of B)

All matmul operands are fp16: same PE streaming rate as fp32r (1 moving
column/cycle, 216 ns per 128x128x512 MM) but half the DMA bytes and
FWL-eligible weight loads. fp8-DoubleRow was probed on hw: 2x FLOPs at the
same 216 ns/MM wall, but e4m3 quantization noise measures ~0.04 max-rel
(vs the 2e-2 gate) and a first-order-corrected operand stack needs 3x
contraction = 1.5x the wall - strictly worse than fp16. fp16 error here
measures ~5e-4.

Schedule (per core):
  phase 1: ob0's d-loop runs first, consuming x k-tiles as they stream in
           (x singles for d<8, 2MB super-tiles after; W-ob0 in 1MB chunks).
  phase 2: u-pass (needs the whole x shard, which has landed by then).
  phase 3: ob0 delta accumulation + packed writeback.
  phase 4: obs 1..7 run o-then-d from a whole-ob resident W (one 4 MB DMA per
           ob, ping-pong prefetched one ob ahead; 32KB contiguous per
           partition row -> large DMA packets). Each (o,t) tile finishes its
           32 W-MMs + fused delta MM back-to-back and drains immediately, so
           writeback is spread evenly and the kernel tail is one tile deep.

Host prep: transposes/tiles x/W/A for contiguous per-partition DMA rows,
builds the one-hot mask. Output y^T returns as fp16, host upcasts.
"""

import numpy as np

from concourse import bacc, tile, mybir
from concourse.bass_utils import run_bass_kernel_spmd
import concourse.bass_utils as _bu

# Disable S3 artifact upload in the trace path (no credentials in this container).
_bu.upload_artifacts = lambda tmpdir: "local://" + tmpdir

N_CORES = 8
T = 8192
D_IN = 4096
D_OUT = 4096
L = 8          # max adapters
R = 16         # max rank
LR = L * R     # 128 = stacked adapter dim
T_SH = T // N_CORES          # 1024 tokens per core
KT = D_IN // 128             # 32 contraction tiles
OB = D_OUT // 512            # 8 output-column superblocks
NO = 4                       # 128-wide output blocks per superblock
NT = T_SH // 512             # 2 token blocks (moving dim)
NSING = 8                    # x k-tiles loaded as singles (startup race)
NSUP = (KT - NSING) // 8     # 3 super-tiles of 8 k-tiles each

F32 = mybir.dt.float32
F16 = mybir.dt.float16

_CACHED_NC = None


def _build():
    nc = bacc.Bacc("TRN2", target_bir_lowering=False, debug=False)

    # x k-tiles 0..NSING-1, row-contiguous [128,1024] singles.
    xT_d = nc.dram_tensor("xT", [NSING * 128, T_SH], F16, kind="ExternalInput")
    # x k-tiles NSING.., packed per super: [p, r*T_SH+t] = x^T[(NSING+8s+r)*128+p, t]
    xs_d = nc.dram_tensor("xsup", [NSUP * 128, 8 * T_SH], F16,
                          kind="ExternalInput")
    # W-ob0: [128, d*512+col] (8KB rows); chunks of 8 d's = 1MB DMAs.
    w0_d = nc.dram_tensor("w0", [128, KT * 512], F16, kind="ExternalInput")
    # W obs1..7: row block ob-1 is [128, d*512+col] (32KB rows); 4MB DMA per ob.
    wob_d = nc.dram_tensor("wob", [(OB - 1) * 128, KT * 512], F16,
                           kind="ExternalInput")
    aT_d = nc.dram_tensor("aTp", [128, KT * LR], F16, kind="ExternalInput")
    bC_d = nc.dram_tensor("bC", [LR, D_OUT], F16, kind="ExternalInput")
    mT_d = nc.dram_tensor("maskT", [LR, T_SH], F16, kind="ExternalInput")
    # y^T [D_OUT, T_SH] fp16 (row-major; per-o128 writeback is contiguous).
    yT_d = nc.dram_tensor("yT", [D_OUT, T_SH], F16, kind="ExternalOutput")

    with tile.TileContext(nc) as tc:
        with (
            tc.tile_pool(name="resident", bufs=1) as rpool,
            tc.tile_pool(name="wzero", bufs=4) as w0pool,
            tc.tile_pool(name="wobp", bufs=2) as wobpool,
            tc.tile_pool(name="yout", bufs=6) as ypool,
            tc.tile_pool(name="psum", bufs=8, space="PSUM") as psum,
        ):
            # --- resident loads: x singles + first w0 chunks first ------------
            xsing = []
            w0c = []
            xsup = []

            def xmov(d, t):
                """moving-operand slice for k-tile d, token block t"""
                if d < NSING:
                    return xsing[d][:, t * 512:(t + 1) * 512]
                s, r = divmod(d - NSING, 8)
                return xsup[s][:, r * T_SH + t * 512: r * T_SH + t * 512 + 512]

            def w0sl(d, o):
                return w0c[d // 8][:, (d % 8) * 512 + o * 128:
                                   (d % 8) * 512 + o * 128 + 128]

            for d in range(2):
                xt = rpool.tile([128, T_SH], F16, tag=f"xt{d}")
                nc.sync.dma_start(xt[:], xT_d[d * 128:(d + 1) * 128, :])
                xsing.append(xt)
            wc = w0pool.tile([128, 8 * 512], F16, tag="w0c", name="w0c0")
            nc.sync.dma_start(wc[:], w0_d[:, 0:8 * 512])
            w0c.append(wc)
            for d in range(2, NSING):
                xt = rpool.tile([128, T_SH], F16, tag=f"xt{d}")
                nc.sync.dma_start(xt[:], xT_d[d * 128:(d + 1) * 128, :])
                xsing.append(xt)
                if d == 4:
                    wc = w0pool.tile([128, 8 * 512], F16, tag="w0c", name="w0c1")
                    nc.sync.dma_start(wc[:], w0_d[:, 8 * 512:16 * 512])
                    w0c.append(wc)
            for s in range(NSUP):
                xs = rpool.tile([128, 8 * T_SH], F16, tag=f"xsup{s}")
                nc.sync.dma_start(xs[:], xs_d[s * 128:(s + 1) * 128, :])
                xsup.append(xs)
                if s < 2:
                    wc = w0pool.tile([128, 8 * 512], F16, tag="w0c",
                                     name=f"w0c{2 + s}")
                    nc.sync.dma_start(
                        wc[:], w0_d[:, (16 + 8 * s) * 512:(24 + 8 * s) * 512])
                    w0c.append(wc)
            atp = rpool.tile([128, KT * LR], F16, tag="atp")
            nc.sync.dma_start(atp[:], aT_d[:])
            bc = rpool.tile([LR, D_OUT], F16, tag="bc")
            nc.sync.dma_start(bc[:], bC_d[:])
            mask = rpool.tile([LR, T_SH], F16, tag="mask")
            nc.sync.dma_start(mask[:], mT_d[:])
            uTms = [rpool.tile([LR, 512], F16, tag=f"uTm{ub}", name=f"uTm{ub}")
                    for ub in range(NT)]
            # prefetch ob1's whole W (needed at ~85us; 4MB, 32KB/row)
            wob_tiles = {}
            wt1 = wobpool.tile([128, KT * 512], F16, tag="wob", name="wob1")
            nc.sync.dma_start(wt1[:], wob_d[0:128, :])
            wob_tiles[1] = wt1

            # --- phase 1: ob0 d-loop (base matmul only, no delta) --------------
            pys0 = [[psum.tile([128, 512], F32, tag="acc", name=f"py0_{o}_{t}")
                     for t in range(NT)] for o in range(NO)]
            yo0s = {}
            for d in range(KT):
                for o in range(NO):
                    for t in range(NT):
                        nc.tensor.matmul(
                            pys0[o][t][:], w0sl(d, o), xmov(d, t),
                            start=(d == 0), stop=(d == KT - 1), skip_group_check=True,
                        )
                        if d == KT - 1:
                            yo0 = rpool.tile([128, 512], F16, tag=f"yo0_{o}_{t}",
                                             name=f"yo0_{o}_{t}")
                            nc.vector.tensor_copy(yo0[:], pys0[o][t][:])
                            yo0s[o, t] = yo0

            # --- phase 2: u-pass (needs all x, which has landed by now) --------
            for ub in range(NT):
                pu = psum.tile([128, 512], F32, tag="acc", name=f"pu{ub}")
                for d in range(KT):
                    nc.tensor.matmul(
                        pu[:], atp[:, d * LR:(d + 1) * LR], xmov(d, ub),
                        start=(d == 0), stop=(d == KT - 1), skip_group_check=True,
                    )
                nc.vector.tensor_mul(uTms[ub][:], pu[:],
                                     mask[:, ub * 512:(ub + 1) * 512])

            # --- phase 3: ob0 delta + packed writeback -------------------------
            for o in range(NO):
                yo = ypool.tile([128, T_SH], F16, tag="yo", name=f"yod{o}")
                for t in range(NT):
                    pd = psum.tile([128, 512], F32, tag="acc", name=f"pd{o}_{t}")
                    nc.tensor.matmul(
                        pd[:], bc[:, o * 128:(o + 1) * 128], uTms[t][:],
                        start=True, stop=True, skip_group_check=True,
                    )
                    nc.vector.tensor_add(yo[:, t * 512:(t + 1) * 512],
                                         yo0s[o, t][:], pd[:])
                nc.sync.dma_start(yT_d[o * 128:(o + 1) * 128, :], yo[:])

            # --- phase 4: obs 1..7, o-then-d from whole-ob resident W ----------
            for ob in range(1, OB):
                if ob + 1 < OB:
                    wnxt = wobpool.tile([128, KT * 512], F16, tag="wob",
                                        name=f"wob{ob + 1}")
                    nc.sync.dma_start(wnxt[:], wob_d[ob * 128:(ob + 1) * 128, :])
                    wob_tiles[ob + 1] = wnxt
                wcur = wob_tiles.pop(ob)
                for o in range(NO):
                    og = ob * 512 + o * 128
                    yo = ypool.tile([128, T_SH], F16, tag="yo", name=f"yo{ob}_{o}")
                    for t in range(NT):
                        py = psum.tile([128, 512], F32, tag="acc",
                                       name=f"py{ob}_{o}_{t}")
                        for d in range(KT):
                            nc.tensor.matmul(
                                py[:], wcur[:, d * 512 + o * 128:
                                            d * 512 + o * 128 + 128],
                                xmov(d, t),
                                start=(d == 0), stop=False, skip_group_check=True,
                            )
                        nc.tensor.matmul(
                            py[:], bc[:, og:og + 128], uTms[t][:],
                            start=False, stop=True, skip_group_check=True,
                        )
                        nc.vector.tensor_copy(yo[:, t * 512:(t + 1) * 512], py[:])
                    nc.sync.dma_start(yT_d[og:og + 128, :], yo[:])

    nc.compile()
    return nc


def _get_nc():
    global _CACHED_NC
    if _CACHED_NC is None:
        _CACHED_NC = _build()
    return _CACHED_NC


def _prep_in_maps(x, weight, lora_A, lora_B, token_to_slot):
    x = np.asarray(x, dtype=np.float32)
    weight = np.asarray(weight, dtype=np.float32)
    lora_A = np.asarray(lora_A, dtype=np.float32)
    lora_B = np.asarray(lora_B, dtype=np.float32)
    slots = np.asarray(token_to_slot)

    # wp[ob, p, d*512+col] = W^T[d*128+p, ob*512+col] = weight[ob*512+col, d*128+p]
    wp = np.ascontiguousarray(
        weight.reshape(OB, 512, KT, 128).transpose(0, 3, 2, 1)
    ).astype(np.float16).reshape(OB, 128, KT * 512)
    w0 = np.ascontiguousarray(wp[0])
    wob = np.ascontiguousarray(wp[1:]).reshape((OB - 1) * 128, KT * 512)

    # aTp[p, d*LR + j] = A-stack^T[d*128+p, j]
    aT = lora_A.transpose(2, 0, 1).reshape(D_IN, LR)           # [D_IN, LR]
    aTp = np.ascontiguousarray(
        aT.reshape(KT, 128, LR).transpose(1, 0, 2)).reshape(128, KT * LR)
    aTp = aTp.astype(np.float16)
    bC = np.ascontiguousarray(
        lora_B.transpose(0, 2, 1).reshape(LR, D_OUT)).astype(np.float16)

    maskT = np.zeros((LR, T), dtype=np.float16)
    for l in range(L):
        maskT[l * R:(l + 1) * R, :] = (slots == l).astype(np.float16)[None, :]

    in_maps = []
    for c in range(N_CORES):
        tsl = slice(c * T_SH, (c + 1) * T_SH)
        xTc = x[tsl, :].T.astype(np.float16)                  # [D_IN, T_SH]
        # supers: [s*128+p, r*T_SH+t] = xTc[(NSING+8s+r)*128+p, t]
        xsup = np.ascontiguousarray(
            xTc[NSING * 128:, :].reshape(NSUP, 8, 128, T_SH)
            .transpose(0, 2, 1, 3)).reshape(NSUP * 128, 8 * T_SH)
        in_maps.append({
            "xT": np.ascontiguousarray(xTc[:NSING * 128, :]),
            "xsup": xsup,
            "w0": w0,
            "wob": wob,
            "aTp": aTp,
            "bC": bC,
            "maskT": np.ascontiguousarray(maskT[:, tsl]),
        })
    return in_maps


def _run(inputs, trace=False, trace_cores=None):
    nc = _get_nc()
    in_maps = _prep_in_maps(**inputs)
    res = run_bass_kernel_spmd(
        nc, in_maps, core_ids=list(range(N_CORES)),
        trace=trace, trace_cores=trace_cores,
    )
    parts = [res.results[c]["yT"].T for c in range(N_CORES)]
    y = np.concatenate(parts, axis=0).astype(np.float32)
    y = np.ascontiguousarray(y)
    return y, res


def _validate(inputs, y):
    """Cheap host-side sanity check: project y onto a random vector and compare
    with the host-computed projection. Catches the (rare, transient) device
    corruption observed on this setup; costs <1 s on host BLAS."""
    x = np.asarray(inputs["x"], dtype=np.float32)
    weight = np.asarray(inputs["weight"], dtype=np.float32)
    lora_A = np.asarray(inputs["lora_A"], dtype=np.float32)
    lora_B = np.asarray(inputs["lora_B"], dtype=np.float32)
    slots = np.asarray(inputs["token_to_slot"])

    rng = np.random.default_rng(12345)
    r = rng.standard_normal(D_OUT).astype(np.float64)

    base = x.astype(np.float64) @ (weight.astype(np.float64).T @ r)      # [T]
    aT = lora_A.transpose(2, 0, 1).reshape(D_IN, LR)                      # [D_IN, LR]
    bC = lora_B.transpose(0, 2, 1).reshape(LR, D_OUT)                     # [LR, D_OUT]
    u = (x @ aT).astype(np.float64)                                       # [T, LR]
    m = np.zeros((T, LR))
    for l in range(L):
        m[:, l * R:(l + 1) * R] = (slots == l).astype(np.float64)[:, None]
    exp = base + (u * m) @ (bC.astype(np.float64) @ r)                    # [T]
    got = y.astype(np.float64) @ r
    scale = np.abs(exp).max()
    rel = np.abs(got - exp).max() / scale
    return rel < 3e-3


def kernel(x, weight, lora_A, lora_B, token_to_slot):
    inputs = dict(x=x, weight=weight, lora_A=lora_A, lora_B=lora_B,
                  token_to_slot=token_to_slot)
    y = None
    for _attempt in range(3):
        y, _ = _run(inputs)
        if _validate(inputs, y):
            break
    return y


# revision 9
# speedup vs baseline: 1.1143x; 1.0038x over previous
"""LoRA row-parallel linear on 8 TRN2 NeuronCores.

Problem: y = x @ W^T + delta, where per-token LoRA delta[t] = B[s] @ (A[s] @ x[t]),
s = token_to_slot[t] (8 adapters, rank 16, scaling baked into B).

Strategy: token data-parallel across the 8 cores (T=8192 -> 1024 tokens/core).
No collectives needed; each core computes its token block fully, in transposed
output space (y^T, un-transposed on the host):
  u^T   = A_all @ x_shard^T          (128 x T_SH; A_all = all 8 adapters stacked)
  uM^T  = u^T * mask^T               (one-hot select of each token's adapter)
  y^T   = W @ x^T + B_all^T @ uM^T   (PSUM accumulation: 32 k-tiles of W + 1 of B)

All matmul operands are fp16: same PE streaming rate as fp32r (1 moving
column/cycle, 216 ns per 128x128x512 MM) but half the DMA bytes and
FWL-eligible weight loads. fp8-DoubleRow was probed on hw: 2x FLOPs at the
same 216 ns/MM wall, but e4m3 quantization noise measures ~0.04 max-rel
(vs the 2e-2 gate) and a first-order-corrected operand stack needs 3x
contraction = 1.5x the wall - strictly worse than fp16. fp16 error here
measures ~5e-4.

Schedule (per core):
  phase 1: ob0's d-loop runs first, consuming x k-tiles as they stream in
           (x singles for d<8, 2MB super-tiles after; W-ob0 in 1MB chunks).
  phase 2: u-pass (needs the whole x shard, which has landed by then).
  phase 3: ob0 delta accumulation + packed writeback.
  phase 4: obs 1..7 run o-then-d from a whole-ob resident W (one 4 MB DMA per
           ob, ping-pong prefetched one ob ahead; 32KB contiguous per
           partition row -> large DMA packets). Each (o,t) tile finishes its
           32 W-MMs + fused delta MM back-to-back and drains immediately, so
           writeback is spread evenly and the kernel tail is one tile deep.

Host prep: transposes/tiles x/W/A for contiguous per-partition DMA rows,
builds the one-hot mask. Output y^T returns as fp16, host upcasts.
"""

import numpy as np
import ml_dtypes

from concourse import bacc, tile, mybir
from concourse.bass_utils import run_bass_kernel_spmd
import concourse.bass_utils as _bu

# Disable S3 artifact upload in the trace path (no credentials in this container).
_bu.upload_artifacts = lambda tmpdir: "local://" + tmpdir

N_CORES = 8
T = 8192
D_IN = 4096
D_OUT = 4096
L = 8          # max adapters
R = 16         # max rank
LR = L * R     # 128 = stacked adapter dim
T_SH = T // N_CORES          # 1024 tokens per core
KT = D_IN // 128             # 32 contraction tiles
OB = D_OUT // 512            # 8 output-column superblocks
NO = 4                       # 128-wide output blocks per superblock
NT = T_SH // 512             # 2 token blocks (moving dim)
NSING = 8                    # x k-tiles loaded as singles (startup race)
NSUP = (KT - NSING) // 8     # 3 super-tiles of 8 k-tiles each

F32 = mybir.dt.float32
F16 = mybir.dt.float16
F8 = mybir.dt.float8e4
KT2 = KT // 2
W0CH = [2, 6, 8, 8, 8]       # ob0 W chunk sizes (in d-tiles); first gate small

_CACHED_NC = None


def _build():
    nc = bacc.Bacc("TRN2", target_bir_lowering=False, debug=False)

    # x k-tiles 0..NSING-1, row-contiguous [128,1024] singles.
    xT_d = nc.dram_tensor("xT", [NSING * 128, T_SH], F16, kind="ExternalInput")
    # x k-tiles NSING.., packed per super: [p, r*T_SH+t] = x^T[(NSING+8s+r)*128+p, t]
    xs_d = nc.dram_tensor("xsup", [NSUP * 128, 8 * T_SH], F16,
                          kind="ExternalInput")
    # W-ob0: [128, d*512+col] (8KB rows); chunks of 8 d's = 1MB DMAs.
    w0_d = nc.dram_tensor("w0", [128, KT * 512], F16, kind="ExternalInput")
    # W obs1..7: row block ob-1 is [128, d*512+col] (32KB rows); 4MB DMA per ob.
    wob_d = nc.dram_tensor("wob", [(OB - 1) * 128, KT * 512], F16,
                           kind="ExternalInput")
    aT_d = nc.dram_tensor("aTp", [128, KT * LR], F16, kind="ExternalInput")
    bC_d = nc.dram_tensor("bC", [LR, D_OUT], F16, kind="ExternalInput")
    mT_d = nc.dram_tensor("maskT", [LR, T_SH], F16, kind="ExternalInput")
    # y^T [D_OUT, T_SH] fp16 (row-major; per-o128 writeback is contiguous).
    yT_d = nc.dram_tensor("yT", [D_OUT, T_SH], F16, kind="ExternalOutput")

    with tile.TileContext(nc) as tc:
        with (
            tc.tile_pool(name="resident", bufs=1) as rpool,
            tc.tile_pool(name="wzero", bufs=1) as w0pool,
            tc.tile_pool(name="wobp", bufs=2) as wobpool,
            tc.tile_pool(name="yout", bufs=6) as ypool,
            tc.tile_pool(name="psum", bufs=8, space="PSUM") as psum,
        ):
            # --- resident loads: x singles + first w0 chunks first ------------
            xsing = []
            w0c = []
            xsup = []

            def xmov(d, t):
                """moving-operand slice for k-tile d, token block t"""
                if d < NSING:
                    return xsing[d][:, t * 512:(t + 1) * 512]
                s, r = divmod(d - NSING, 8)
                return xsup[s][:, r * T_SH + t * 512: r * T_SH + t * 512 + 512]

            def w0sl(d, o):
                ci = max(i for i in range(len(W0CH)) if w0starts[i] <= d)
                r = (d - w0starts[ci]) * 512 + o * 128
                return w0c[ci][:, r:r + 128]

            w0starts = [sum(W0CH[:i]) for i in range(len(W0CH))]

            def load_w0c(ci):
                nd = W0CH[ci]
                wc = w0pool.tile([128, nd * 512], F16, tag=f"w0c{ci}",
                                 name=f"w0c{ci}")
                c0 = w0starts[ci] * 512
                nc.sync.dma_start(wc[:], w0_d[:, c0:c0 + nd * 512])
                w0c.append(wc)

            for d in range(2):
                xt = rpool.tile([128, T_SH], F16, tag=f"xt{d}")
                nc.sync.dma_start(xt[:], xT_d[d * 128:(d + 1) * 128, :])
                xsing.append(xt)
                if d == 0:
                    load_w0c(0)
            for d in range(2, NSING):
                xt = rpool.tile([128, T_SH], F16, tag=f"xt{d}")
                nc.sync.dma_start(xt[:], xT_d[d * 128:(d + 1) * 128, :])
                xsing.append(xt)
                if d == 2:
                    load_w0c(1)
                if d == 6:
                    load_w0c(2)
            for s in range(NSUP):
                xs = rpool.tile([128, 8 * T_SH], F16, tag=f"xsup{s}")
                nc.sync.dma_start(xs[:], xs_d[s * 128:(s + 1) * 128, :])
                xsup.append(xs)
                if s < 2:
                    load_w0c(3 + s)
            bc = rpool.tile([LR, D_OUT], F16, tag="bc")
            nc.sync.dma_start(bc[:], bC_d[:])
            mask = rpool.tile([LR, T_SH], F16, tag="mask")
            nc.sync.dma_start(mask[:], mT_d[:])
            uTms = [rpool.tile([LR, 512], F16, tag=f"uTm{ub}", name=f"uTm{ub}")
                    for ub in range(NT)]
            atp = rpool.tile([128, KT * LR], F16, tag="atp")
            nc.sync.dma_start(atp[:], aT_d[:])
            # prefetch ob1's whole W (needed at ~85us; 4MB, 32KB/row)
            wob_tiles = {}
            wt1 = wobpool.tile([128, KT * 512], F16, tag="wob", name="wob1")
            nc.sync.dma_start(wt1[:], wob_d[0:128, :])
            wob_tiles[1] = wt1

            # --- phase 1: ob0 d-loop (base matmul only, no delta) --------------
            pys0 = [[psum.tile([128, 512], F32, tag="acc", name=f"py0_{o}_{t}")
                     for t in range(NT)] for o in range(NO)]
            yo0s = {}
            for d in range(KT):
                for o in range(NO):
                    for t in range(NT):
                        nc.tensor.matmul(
                            pys0[o][t][:], w0sl(d, o), xmov(d, t),
                            start=(d == 0),
                            stop=(d == KT - 1 and o == NO - 1),
                            skip_group_check=True,
                        )
                        if d == KT - 1 and o == NO - 1:
                            yo0 = rpool.tile([128, 512], F16, tag=f"yo0_{o}_{t}",
                                             name=f"yo0_{o}_{t}")
                            nc.vector.tensor_copy(yo0[:], pys0[o][t][:])
                            yo0s[o, t] = yo0

            # --- phase 2: u-pass (needs all x, which has landed by now) --------
            for ub in range(NT):
                pu = psum.tile([128, 512], F32, tag="acc", name=f"pu{ub}")
                sl = slice(ub * 512, (ub + 1) * 512)
                for d in range(KT):
                    nc.tensor.matmul(
                        pu[:], atp[:, d * LR:(d + 1) * LR], xmov(d, ub),
                        start=(d == 0), stop=(d == KT - 1),
                        skip_group_check=True,
                    )
                nc.vector.tensor_mul(uTms[ub][:], pu[:],
                                     mask[:, ub * 512:(ub + 1) * 512])

            # --- phase 3: ob0 delta + packed writeback -------------------------
            for o in range(NO):
                yo = ypool.tile([128, T_SH], F16, tag="yo", name=f"yod{o}")
                for t in range(NT):
                    if o < NO - 1:
                        nc.tensor.matmul(
                            pys0[o][t][:], bc[:, o * 128:(o + 1) * 128],
                            uTms[t][:],
                            start=False, stop=True, skip_group_check=True,
                        )
                        nc.vector.tensor_copy(yo[:, t * 512:(t + 1) * 512],
                                              pys0[o][t][:])
                    else:
                        pd = psum.tile([128, 512], F32, tag="acc",
                                       name=f"pd{o}_{t}")
                        nc.tensor.matmul(
                            pd[:], bc[:, o * 128:(o + 1) * 128], uTms[t][:],
                            start=True, stop=True, skip_group_check=True,
                        )
                        nc.vector.tensor_add(yo[:, t * 512:(t + 1) * 512],
                                             yo0s[o, t][:], pd[:])
                nc.sync.dma_start(yT_d[o * 128:(o + 1) * 128, :], yo[:])

            # --- phase 4: obs 1..7, o-then-d from whole-ob resident W ----------
            for ob in range(1, OB):
                if ob + 1 < OB:
                    wnxt = wobpool.tile([128, KT * 512], F16, tag="wob",
                                        name=f"wob{ob + 1}")
                    nc.sync.dma_start(wnxt[:], wob_d[ob * 128:(ob + 1) * 128, :])
                    wob_tiles[ob + 1] = wnxt
                wcur = wob_tiles.pop(ob)
                for o in range(NO):
                    og = ob * 512 + o * 128
                    yo = ypool.tile([128, T_SH], F16, tag="yo", name=f"yo{ob}_{o}")
                    for t in range(NT):
                        py = psum.tile([128, 512], F32, tag="acc",
                                       name=f"py{ob}_{o}_{t}")
                        for d in range(KT):
                            nc.tensor.matmul(
                                py[:], wcur[:, d * 512 + o * 128:
                                            d * 512 + o * 128 + 128],
                                xmov(d, t),
                                start=(d == 0), stop=False, skip_group_check=True,
                            )
                        nc.tensor.matmul(
                            py[:], bc[:, og:og + 128], uTms[t][:],
                            start=False, stop=True, skip_group_check=True,
                        )
                        nc.vector.tensor_copy(yo[:, t * 512:(t + 1) * 512], py[:])
                    nc.sync.dma_start(yT_d[og:og + 128, :], yo[:])

    nc.compile()
    return nc


def _get_nc():
    global _CACHED_NC
    if _CACHED_NC is None:
        _CACHED_NC = _build()
    return _CACHED_NC


def _prep_in_maps(x, weight, lora_A, lora_B, token_to_slot):
    x = np.asarray(x, dtype=np.float32)
    weight = np.asarray(weight, dtype=np.float32)
    lora_A = np.asarray(lora_A, dtype=np.float32)
    lora_B = np.asarray(lora_B, dtype=np.float32)
    slots = np.asarray(token_to_slot)

    # wp[ob, p, d*512+col] = W^T[d*128+p, ob*512+col] = weight[ob*512+col, d*128+p]
    wp = np.ascontiguousarray(
        weight.reshape(OB, 512, KT, 128).transpose(0, 3, 2, 1)
    ).astype(np.float16).reshape(OB, 128, KT * 512)
    w0 = np.ascontiguousarray(wp[0])
    wob = np.ascontiguousarray(wp[1:]).reshape((OB - 1) * 128, KT * 512)

    aT = lora_A.transpose(2, 0, 1).reshape(D_IN, LR)           # [D_IN, LR]
    aTp = np.ascontiguousarray(
        aT.reshape(KT, 128, LR).transpose(1, 0, 2)).reshape(128, KT * LR)
    aTp = aTp.astype(np.float16)
    bC = np.ascontiguousarray(
        lora_B.transpose(0, 2, 1).reshape(LR, D_OUT)).astype(np.float16)

    maskT = np.zeros((LR, T), dtype=np.float16)
    for l in range(L):
        maskT[l * R:(l + 1) * R, :] = (slots == l).astype(np.float16)[None, :]

    in_maps = []
    for c in range(N_CORES):
        tsl = slice(c * T_SH, (c + 1) * T_SH)
        xTc = x[tsl, :].T.astype(np.float16)                  # [D_IN, T_SH]
        # supers: [s*128+p, r*T_SH+t] = xTc[(NSING+8s+r)*128+p, t]
        xsup = np.ascontiguousarray(
            xTc[NSING * 128:, :].reshape(NSUP, 8, 128, T_SH)
            .transpose(0, 2, 1, 3)).reshape(NSUP * 128, 8 * T_SH)
        in_maps.append({
            "aTp": aTp,
            "xT": np.ascontiguousarray(xTc[:NSING * 128, :]),
            "xsup": xsup,
            "w0": w0,
            "wob": wob,
            "bC": bC,
            "maskT": np.ascontiguousarray(maskT[:, tsl]),
        })
    return in_maps


def _run(inputs, trace=False, trace_cores=None):
    nc = _get_nc()
    in_maps = _prep_in_maps(**inputs)
    res = run_bass_kernel_spmd(
        nc, in_maps, core_ids=list(range(N_CORES)),
        trace=trace, trace_cores=trace_cores,
    )
    parts = [res.results[c]["yT"].T for c in range(N_CORES)]
    y = np.concatenate(parts, axis=0).astype(np.float32)
    y = np.ascontiguousarray(y)
    return y, res


def _validate(inputs, y):
    """Cheap host-side sanity check: project y onto a random vector and compare
    with the host-computed projection. Catches the (rare, transient) device
    corruption observed on this setup; costs <1 s on host BLAS."""
    x = np.asarray(inputs["x"], dtype=np.float32)
    weight = np.asarray(inputs["weight"], dtype=np.float32)
    lora_A = np.asarray(inputs["lora_A"], dtype=np.float32)
    lora_B = np.asarray(inputs["lora_B"], dtype=np.float32)
    slots = np.asarray(inputs["token_to_slot"])

    rng = np.random.default_rng(12345)
    r = rng.standard_normal(D_OUT).astype(np.float64)

    base = x.astype(np.float64) @ (weight.astype(np.float64).T @ r)      # [T]
    aT = lora_A.transpose(2, 0, 1).reshape(D_IN, LR)                      # [D_IN, LR]
    bC = lora_B.transpose(0, 2, 1).reshape(LR, D_OUT)                     # [LR, D_OUT]
    u = (x @ aT).astype(np.float64)                                       # [T, LR]
    m = np.zeros((T, LR))
    for l in range(L):
        m[:, l * R:(l + 1) * R] = (slots == l).astype(np.float64)[:, None]
    exp = base + (u * m) @ (bC.astype(np.float64) @ r)                    # [T]
    got = y.astype(np.float64) @ r
    scale = np.abs(exp).max()
    rel = np.abs(got - exp).max() / scale
    return rel < 3e-3


def kernel(x, weight, lora_A, lora_B, token_to_slot):
    inputs = dict(x=x, weight=weight, lora_A=lora_A, lora_B=lora_B,
                  token_to_slot=token_to_slot)
    y = None
    for _attempt in range(3):
        y, _ = _run(inputs)
        if _validate(inputs, y):
            break
    return y
